# revision 31
# baseline (speedup 1.0000x reference)
"""Trainium2 Bass kernel for nn_Attention_57166014709861.

8-batch image attention (B=8, C=384, h=8, d=48, HW=1024), data-parallel:
one batch image per NeuronCore, weights broadcast, host-side gather.
HW exec ~109.4us (prior session's f32r baseline: 139us; serial: 222us).

Everything except the f32 softmax denominators runs in bf16 (rel err
5.6e-3 vs the 2e-2 gate). Inputs pre-laid-out host-side; x is packed
j-half-major [P, 2(j), 3(ko), 512] so each half loads as one 3KB-line DMA.

Per-core pipeline:
  prelude: only q,k tile 0 (12 matmuls, j-interleaved with half-tile DVE
         casts) before the pair loop, so the ACT exp stream starts ~19us
         in (was 38.7us).
  qkv:   v chunks (one per yt) and q,k tile 1 are emitted INSIDE pair 0's
         yt loop; q,k tile 2 inside pair 1; tile 3 inside pair 2 — each
         pair's PE work (~15-18us) balances its ACT exp work (16.5us).
         v computed transposed as vT [seq, packed-c] with a leading ones
         column per head so the softmax denominator rides the av matmul
         (psum rows 0 / 64).
  attn:  per head pair, interleaved through the y-tile loop with a
         pair-shared av psum tile (rows 0:64 / 64:128, av j-major so the
         j0 half stops first) and THREE sT psum slots: sT[y,x] = k_h^T q_h
         (bf16, scale folded into wq) -> exp on ACT -> av accumulates
         u'[c,x] + denominator over y tiles in PSUM. ACT is ~gapless
         through pairs 1-3; PE is the binding stream (~80us busy).
  norm (pairs 0-2): denominator rows DMA-transposed to [128,16] via a DRAM
         bounce (DVE reciprocal is per-lane serial; staged yt2/yt3/yt5 of
         the next pair so the DVE queue never parks on a DMA wait),
         reciprocal in bf16, 1/den broadcast back with two stride-0 DMA
         reads (one per queue), one DVE multiply; off the critical path.
  norm (final pair): per j-half pipelined — av copy halves, den rows
         scatter-DMA'd to [128,8], DVE reciprocal, rows gathered to
         [2,512], then a K=2 selector MATMUL against a host-packed 0/1
         matrix broadcasts 1/den across partitions into PSUM (replaces the
         ~8us stride-0 broadcast DMA), one DVE multiply per half. Proj
         kt0-1 runs on the PE during this chain; kt2 fills the gap between
         the two selector matmuls.
  proj:  w_projT packed on K with zero pad rows (bf16); kt3 lands last per
         j-half; bias added during psum->sbuf copy on ACT (ot0/ot2) and
         DVE (ot1) in parallel; six bf16 output DMAs rotate across three
         queues; host casts back to f32.
"""

import sys

if "/opt/trn_rl_repo" not in sys.path:
    sys.path.insert(0, "/opt/trn_rl_repo")

import numpy as np

import concourse.bass as bass
import concourse.mybir as mybir
import concourse.tile as tile
from concourse import bacc
from concourse.bass_utils import run_bass_kernel_spmd

DIM = 384
HEADS = 8
DH = 48
SEQ = 1024
P = 128
NCORES = 8
VP = 64  # packed v cols per head: ones at col 0, zeros 1-15, 48 data at 16-63

F32 = mybir.dt.float32
F32R = mybir.dt.float32r
BF16 = mybir.dt.bfloat16
EXP = mybir.ActivationFunctionType.Exp
IDENT = mybir.ActivationFunctionType.Identity
ADD = mybir.AluOpType.add
MULT = mybir.AluOpType.mult

_NC_CACHE = {}


def _emit(tc, nc, x_d, wq_d, wk_d, wv_d, wp_d, b_d, sel2_d, out_d):
    with (
        tc.tile_pool(name="const", bufs=1) as constp,
        tc.tile_pool(name="weights", bufs=1) as wpool,
        tc.tile_pool(name="data", bufs=1) as data,
        tc.tile_pool(name="ptile", bufs=8) as ppool,
        tc.tile_pool(name="bcpool", bufs=3) as bcpool,
        tc.tile_pool(name="rpool", bufs=3) as rpool,
        tc.tile_pool(name="avcp", bufs=2) as avcp,
        tc.tile_pool(name="opool", bufs=3) as opool,
        tc.tile_pool(name="ps_s", bufs=3, space="PSUM") as ps_s,
        tc.tile_pool(name="ps_av", bufs=1, space="PSUM") as ps_av,
        tc.tile_pool(name="dram", bufs=3, space="DRAM") as drampool,
    ):
        # ---- loads, ordered by need: x + wq/wk tile-0 cols gate the start
        x_sb = data.tile([P, 2, 3, 512], BF16, tag="x")
        wq_sb = wpool.tile([P, 3, 512], BF16, tag="wq")
        wk_sb = wpool.tile([P, 3, 512], BF16, tag="wk")
        wv_sb = wpool.tile([P, 3, HEADS * VP], BF16, tag="wv")
        wp_sb = wpool.tile([P, 4, DIM], BF16, tag="wp")
        bias_sb = constp.tile([P, 3], F32, tag="bias")

        # x packed j-half-major: each half is one 384KB DMA with 3KB lines.
        # wv/wp/wq-rest/wk-rest are queued BEHIND the x halves so their
        # ~1.3MB doesn't steal DMA-engine bandwidth from the loads that gate
        # the first matmul (wv is needed at ~22us, wp only at ~92us).
        nc.sync.dma_start(x_sb[:, 0], x_d.ap()[:, 0])
        nc.scalar.dma_start(x_sb[:, 1], x_d.ap()[:, 1])
        nc.gpsimd.dma_start(wq_sb[:, :, 0:128], wq_d.ap()[:, :, 0:128])
        nc.gpsimd.dma_start(wk_sb[:, :, 0:128], wk_d.ap()[:, :, 0:128])
        nc.gpsimd.dma_start(bias_sb[:], b_d.ap())
        nc.sync.dma_start(wq_sb[:, :, 128:512], wq_d.ap()[:, :, 128:512])
        nc.scalar.dma_start(wk_sb[:, :, 128:512], wk_d.ap()[:, :, 128:512])
        nc.sync.dma_start(wv_sb[:], wv_d.ap())
        nc.scalar.dma_start(wp_sb[:], wp_d.ap())

        zb_sb = constp.tile([P, 1], F32, tag="zb")
        nc.gpsimd.memset(zb_sb[:], 0.0)
        # selector for the final-pair 1/den broadcast matmul (host-packed):
        # bc[p, x] = sum_k sel2[k, p] * rec2[k, x] -> rows 0:64 get rec2[0],
        # rows 64:128 get rec2[1]
        sel2_sb = constp.tile([2, P], BF16, tag="sel2")
        nc.gpsimd.dma_start(sel2_sb[:], sel2_d.ap())

        q_sb = data.tile([P, 4, SEQ], BF16, tag="q")
        k_sb = data.tile([P, 4, SEQ], BF16, tag="k")
        vT_sb = data.tile([P, 8, HEADS, VP], BF16, tag="vT")
        u_sb = [data.tile([P, SEQ], BF16, tag=f"u{i}", name=f"u{i}") for i in range(4)]

        def emit_qk_tile(t, which):
            """6 matmuls + 1 cast: one q or k tile into bf16 SBUF."""
            dst, w = (q_sb, wq_sb) if which == "q" else (k_sb, wk_sb)
            ps = ps_s.tile([P, SEQ], F32, tag="s", name=f"{which}{t}_ps")
            for j in range(2):
                for ko in range(3):
                    nc.tensor.matmul(
                        ps[:, j * 512 : (j + 1) * 512],
                        lhsT=w[:, ko, t * 128 : (t + 1) * 128],
                        rhs=x_sb[:, j, ko, :],
                        start=(ko == 0),
                        stop=(ko == 2),
                    )
            nc.vector.tensor_copy(dst[:, t, :], ps[:])

        def emit_v_chunk(yt):
            """3 matmuls + copy + ones-column: vT rows for one y tile."""
            ps = ps_s.tile([P, SEQ], F32, tag="s", name="v_ps")
            for ko in range(3):
                nc.tensor.matmul(
                    ps[:, 0 : HEADS * VP],
                    lhsT=x_sb[:, yt // 4, ko, (yt % 4) * 128 : (yt % 4 + 1) * 128],
                    rhs=wv_sb[:, ko, :],
                    start=(ko == 0),
                    stop=(ko == 2),
                )
            nc.vector.tensor_copy(
                vT_sb[:, yt, :, :],
                ps[:, 0 : HEADS * VP].rearrange("p (h v) -> p h v", h=HEADS),
            )
            nc.gpsimd.memset(vT_sb[:, yt, :, 0:1], 1.0)

        # prelude: only q,k tile 0 before the pair loop, j-interleaved with
        # half-tile casts so sT(pair0, yt0) unblocks as early as possible
        q0_ps = ps_s.tile([P, SEQ], F32, tag="s", name="q0_ps")
        k0_ps = ps_s.tile([P, SEQ], F32, tag="s", name="k0_ps")
        for j in range(2):
            for ps, w, dst in ((q0_ps, wq_sb, q_sb), (k0_ps, wk_sb, k_sb)):
                for ko in range(3):
                    nc.tensor.matmul(
                        ps[:, j * 512 : (j + 1) * 512],
                        lhsT=w[:, ko, 0:128],
                        rhs=x_sb[:, j, ko, :],
                        start=(ko == 0),
                        stop=(ko == 2),
                    )
                nc.vector.tensor_copy(
                    dst[:, 0, j * 512 : (j + 1) * 512],
                    ps[:, j * 512 : (j + 1) * 512],
                )

        def epi_a(t, av_copy, q=None):
            # av_copy rows 0/64 = softmax denominators of heads 2t/2t+1.
            # DMA-transpose both to [128, 16] via a DRAM bounce so the
            # reciprocal runs on 128 lanes.
            q = q or nc.sync
            den_dram = drampool.tile([2 * SEQ], F32, tag="den")
            q.dma_start(den_dram[0:SEQ], av_copy[0:1, :])
            q.dma_start(den_dram[SEQ : 2 * SEQ], av_copy[64:65, :])
            den_pm = rpool.tile([P, 16], F32, tag="denpm")
            q.dma_start(den_pm[:], den_dram[:].rearrange("(p f) -> p f", p=P))
            rec_pm = rpool.tile([P, 16], BF16, tag="recpm")
            with nc.allow_low_precision(reason="softmax denom reciprocal to bf16"):
                nc.vector.reciprocal(rec_pm[:], den_pm[:])
            rec_dram = drampool.tile([2 * SEQ], BF16, tag="rec")
            q.dma_start(rec_dram[:], rec_pm[:])
            return rec_dram

        def epi_b_issue(t, av_copy, rec_dram):
            # stride-0 DMAs broadcast both heads' 1/denom to [128, 1024]
            # (rows 0:64 <- head 2t, rows 64:128 <- head 2t+1), one per queue
            # so the ~256KB broadcast lands in half the time.
            bc_sb = bcpool.tile([P, SEQ], BF16, tag="bcs")
            rec_r = rec_dram[:].rearrange("(h f) -> h f", h=2)
            nc.sync.dma_start(
                bc_sb[0:64, :], rec_r[0:1, None, :].to_broadcast([1, 64, SEQ])
            )
            nc.scalar.dma_start(
                bc_sb[64:128, :], rec_r[1:2, None, :].to_broadcast([1, 64, SEQ])
            )
            return bc_sb

        def epi_b_mult(t, av_copy, bc_sb):
            # emitted well after the broadcast issue so this never parks the
            # in-order DVE queue on a DMA wait.
            with nc.allow_low_precision(reason="bf16 u tile"):
                nc.vector.tensor_tensor(u_sb[t][:], av_copy[:], bc_sb[:], MULT)

        # Head-pair interleaved attention with qkv work spread through the
        # yt loops so the PE load per pair tracks the 16.5us ACT exp load.
        pending = []
        last_av = None
        for t in range(4):
            av_ps = ps_av.tile([P, SEQ], F32, tag="av", name=f"av{t}")
            p_tiles = [[None] * 8 for _ in range(2)]
            for yt in range(9):
                for s in range(2):
                    po = s * 64
                    if yt < 8:
                        sT_ps = ps_s.tile([P, SEQ], F32, tag="s", name="sT_ps")
                        for j in range(2):
                            nc.tensor.matmul(
                                sT_ps[:, j * 512 : (j + 1) * 512],
                                lhsT=k_sb[
                                    po : po + 48, t, yt * 128 : (yt + 1) * 128
                                ],
                                rhs=q_sb[po : po + 48, t, j * 512 : (j + 1) * 512],
                                start=True,
                                stop=True,
                            )
                        p_tiles[s][yt] = ppool.tile(
                            [P, SEQ], BF16, tag="p", name="p_sb"
                        )
                        nc.scalar.activation(
                            p_tiles[s][yt][:], sT_ps[:], EXP, bias=zb_sb[:]
                        )
                if yt > 0:
                    for j in range(2):
                        for s in range(2):
                            po = s * 64
                            h = 2 * t + s
                            nc.tensor.matmul(
                                av_ps[po : po + VP, j * 512 : (j + 1) * 512],
                                lhsT=vT_sb[:, yt - 1, h, :],
                                rhs=p_tiles[s][yt - 1][:, j * 512 : (j + 1) * 512],
                                start=(yt == 1),
                                stop=(yt == 8),
                                skip_group_check=True,
                            )
                # qkv spread: v chunks ride pair 0; q/k tile t+1 rides pair t
                if t == 0 and yt < 8:
                    emit_v_chunk(yt)
                if t < 3:
                    if yt == 2:
                        emit_qk_tile(t + 1, "q")
                    elif yt == 4:
                        emit_qk_tile(t + 1, "k")
                # Epilogue staging (pair t-1, hosted early in pair t as in
                # the fast v2.3 schedule): reciprocal chain at yt2, broadcast
                # DMA issue at yt3, multiply at yt5 — the two-yt offset keeps
                # the in-order DVE queue from parking on the DMA wait.
                if yt == 2 and pending and pending[0][-1] == "avc":
                    tt, avc, _ = pending[0]
                    pending[0] = (tt, avc, epi_a(tt, avc), "rec")
                if yt == 3 and pending and pending[0][-1] == "rec":
                    tt, avc, rec, _ = pending[0]
                    pending[0] = (tt, avc, epi_b_issue(tt, avc, rec), "bc")
                if yt == 5 and pending and pending[0][-1] == "bc":
                    epi_b_mult(*pending.pop(0)[:3])
            av_copy = avcp.tile([P, SEQ], F32, tag="avc", name=f"avc{t}")
            if t < 3:
                nc.vector.tensor_copy(av_copy[:], av_ps[:])
                pending.append((t, av_copy, "avc"))
            else:
                last_av = (av_ps, av_copy)

        # ---- final pair epilogue, pipelined in j-halves ----
        # Per half: av copy out of PSUM, den rows scatter-transposed to
        # [128, 8] (rows 0:64 <- head A half, 64:128 <- head B half), DVE
        # reciprocal, rows gathered to [2, 512], K=2 selector matmul
        # broadcasts 1/den across partitions, one multiply. The j1 chain
        # runs one stage behind j0, and proj kt0-2 fills the PE meanwhile.
        av_ps3, av_copy3 = last_av
        halves = []
        for jh in range(2):
            sl = slice(jh * 512, (jh + 1) * 512)
            nc.vector.tensor_copy(av_copy3[:, sl], av_ps3[:, sl])
            den_pm = rpool.tile([P, 8], F32, tag="denpm", name=f"denpm{jh}")
            nc.sync.dma_start(den_pm[0:64, :], av_copy3[0:1, sl])
            nc.scalar.dma_start(den_pm[64:128, :], av_copy3[64:65, sl])
            halves.append((sl, den_pm))

        # proj kt0-1 for all three output tiles keeps the PE busy while the
        # final pair's 1/den chain resolves (emitted after av_copy so ot2 can
        # reuse the ps_av bank); kt2 is held back and emitted between the two
        # bc matmuls so the PE never idles (and never drops p-state) while
        # the rec2 gather DMAs land.
        pr_ps = []

        def emit_pr(kts):
            for ot in range(3):
                for j in range(2):
                    for kt in kts:
                        nc.tensor.matmul(
                            pr_ps[ot][:, j * 512 : (j + 1) * 512],
                            lhsT=wp_sb[:, kt, ot * 128 : (ot + 1) * 128],
                            rhs=u_sb[kt][:, j * 512 : (j + 1) * 512],
                            start=(kt == 0),
                            stop=False,
                        )

        for ot in range(3):
            pool = ps_s if ot < 2 else ps_av
            pr_ps.append(
                pool.tile(
                    [P, SEQ], F32, tag=("s" if ot < 2 else "av"), name=f"prps{ot}"
                )
            )
        emit_pr([0, 1])

        bc_ps = ps_s.tile([P, SEQ], F32, tag="s", name="bc_ps")
        for jh, (sl, den_pm) in enumerate(halves):
            rec_pm = rpool.tile([P, 8], BF16, tag="recpm", name=f"recpm{jh}")
            with nc.allow_low_precision(reason="softmax denom reciprocal to bf16"):
                nc.vector.reciprocal(rec_pm[:], den_pm[:])
            rec2_sb = rpool.tile([2, 512], BF16, tag="rec2", name=f"rec2_{jh}")
            nc.sync.dma_start(rec2_sb[0:1, :], rec_pm[0:64, :])
            nc.scalar.dma_start(rec2_sb[1:2, :], rec_pm[64:128, :])
            nc.tensor.matmul(
                bc_ps[:, sl],
                lhsT=sel2_sb[:, :],
                rhs=rec2_sb[:, :],
                start=True,
                stop=True,
            )
            if jh == 0:
                emit_pr([2])
            with nc.allow_low_precision(reason="bf16 u tile"):
                nc.vector.tensor_tensor(
                    u_sb[3][:, sl], av_copy3[:, sl], bc_ps[:, sl], MULT
                )

        # ---- proj kt3 + bias + output DMAs, per (ot, j-half) ----
        # ot1's bias add goes to the (idle) DVE so psum evacuations run on
        # two engines; six 128KB output DMAs rotate across three queues.
        oqs = [nc.sync, nc.scalar, nc.gpsimd]
        o_sbs = [opool.tile([P, SEQ], BF16, tag="o", name=f"o{ot}") for ot in range(3)]
        for jh in range(2):
            sl = slice(jh * 512, (jh + 1) * 512)
            for ot in range(3):
                nc.tensor.matmul(
                    pr_ps[ot][:, sl],
                    lhsT=wp_sb[:, 3, ot * 128 : (ot + 1) * 128],
                    rhs=u_sb[3][:, sl],
                    start=False,
                    stop=True,
                )
            for ot in range(3):
                o_sb = o_sbs[ot]
                if ot == 1:
                    with nc.allow_low_precision(reason="bf16 output tile"):
                        nc.vector.tensor_scalar(
                            o_sb[:, sl],
                            pr_ps[ot][:, sl],
                            bias_sb[:, ot : ot + 1],
                            None,
                            ADD,
                        )
                else:
                    nc.scalar.activation(
                        o_sb[:, sl],
                        pr_ps[ot][:, sl],
                        IDENT,
                        bias=bias_sb[:, ot : ot + 1],
                    )
                oqs[ot].dma_start(
                    out_d.ap()[ot * 128 : (ot + 1) * 128, sl], o_sb[:, sl]
                )


def build_nc():
    nc = bacc.Bacc("TRN2", target_bir_lowering=False, debug=False, num_devices=NCORES)
    x_d = nc.dram_tensor("x", [P, 2, 3, 512], BF16, kind="ExternalInput")
    wq_d = nc.dram_tensor("wq", [P, 3, 512], BF16, kind="ExternalInput")
    wk_d = nc.dram_tensor("wk", [P, 3, 512], BF16, kind="ExternalInput")
    wv_d = nc.dram_tensor("wv", [P, 3, HEADS * VP], BF16, kind="ExternalInput")
    wp_d = nc.dram_tensor("wp", [P, 4, DIM], BF16, kind="ExternalInput")
    b_d = nc.dram_tensor("bias", [P, 3], F32, kind="ExternalInput")
    sel2_d = nc.dram_tensor("sel2", [2, P], BF16, kind="ExternalInput")
    out_d = nc.dram_tensor("out", [DIM, SEQ], BF16, kind="ExternalOutput")

    with tile.TileContext(nc) as tc:
        _emit(tc, nc, x_d, wq_d, wk_d, wv_d, wp_d, b_d, sel2_d, out_d)
    nc.compile()
    return nc


import ml_dtypes

BF16NP = ml_dtypes.bfloat16


def pack_inputs(x, w_qkv, w_proj, b_proj):
    """Host-side weight packing. Returns per-core input maps."""
    x = np.asarray(x, np.float32)
    w_qkv = np.asarray(w_qkv, np.float32)
    w_proj = np.asarray(w_proj, np.float32)
    b_proj = np.asarray(b_proj, np.float32)
    scale = DH ** -0.5
    w_q, w_k, w_v = w_qkv[0:DIM], w_qkv[DIM : 2 * DIM], w_qkv[2 * DIM :]

    WQ = np.zeros((DIM, 512), np.float32)
    WK = np.zeros((DIM, 512), np.float32)
    WV = np.zeros((DIM, HEADS * VP), np.float32)
    WP = np.zeros((512, DIM), np.float32)
    for h in range(HEADS):
        col = (h // 2) * 128 + (h % 2) * 64
        WQ[:, col : col + DH] = (w_q[h * DH : (h + 1) * DH] * scale).T
        WK[:, col : col + DH] = w_k[h * DH : (h + 1) * DH].T
        WV[:, h * VP + 16 : h * VP + 16 + DH] = w_v[h * DH : (h + 1) * DH].T
        WP[col + 16 : col + 16 + DH, :] = w_proj[:, h * DH : (h + 1) * DH].T
    BIAS = np.ascontiguousarray(b_proj.reshape(3, P).T)

    def pm(a, chunks):
        # [(chunks*P), f] -> [P, chunks, f] partition-major pre-layout
        return np.ascontiguousarray(
            a.reshape(chunks, P, a.shape[-1]).transpose(1, 0, 2)
        )

    def pmb(a, chunks):
        return pm(a, chunks).astype(BF16NP)

    def xj(a):
        # [DIM, SEQ] -> [P, 2(j-half), 3(ko), 512] j-half-major bf16
        xp = pm(a, 3)  # [P, 3, SEQ]
        xp = xp.reshape(P, 3, 2, 512).transpose(0, 2, 1, 3)
        return np.ascontiguousarray(xp).astype(BF16NP)

    WQp, WKp, WVp, WPp = pmb(WQ, 3), pmb(WK, 3), pmb(WV, 3), pmb(WP, 4)
    SEL2 = np.zeros((2, P), np.float32)
    SEL2[0, 0:64] = 1.0
    SEL2[1, 64:128] = 1.0
    SEL2 = SEL2.astype(BF16NP)
    in_maps = []
    for b in range(NCORES):
        in_maps.append(
            {
                "x": xj(x[b].reshape(DIM, SEQ)),
                "wq": WQp,
                "wk": WKp,
                "wv": WVp,
                "wp": WPp,
                "bias": BIAS,
                "sel2": SEL2,
            }
        )
    return in_maps


def run(in_maps, trace=False):
    if "nc" not in _NC_CACHE:
        _NC_CACHE["nc"] = build_nc()
    nc = _NC_CACHE["nc"]
    res = run_bass_kernel_spmd(
        nc, in_maps, core_ids=list(range(NCORES)), trace=trace
    )
    out = np.stack(
        [np.asarray(res.results[i]["out"], np.float32) for i in range(NCORES)]
    )
    return out.reshape(NCORES, DIM, 32, 32), res


def kernel(x, w_qkv, w_proj, b_proj):
    out, _ = run(pack_inputs(x, w_qkv, w_proj, b_proj))
    return out


# revision 32
# speedup vs baseline: 1.0496x; 1.0496x over previous
"""Trainium2 Bass kernel for nn_Attention_57166014709861.

8-batch image attention (B=8, C=384, h=8, d=48, HW=1024), data-parallel:
one batch image per NeuronCore, weights broadcast, host-side gather.
HW exec ~109.4us (prior session's f32r baseline: 139us; serial: 222us).

Everything except the f32 softmax denominators runs in bf16 (rel err
5.6e-3 vs the 2e-2 gate). Inputs pre-laid-out host-side; x is packed
j-half-major [P, 2(j), 3(ko), 512] so each half loads as one 3KB-line DMA.

Per-core pipeline:
  prelude: only q,k tile 0 (12 matmuls, j-interleaved with half-tile DVE
         casts) before the pair loop, so the ACT exp stream starts ~19us
         in (was 38.7us).
  qkv:   v chunks (one per yt) and q,k tile 1 are emitted INSIDE pair 0's
         yt loop; q,k tile 2 inside pair 1; tile 3 inside pair 2 — each
         pair's PE work (~15-18us) balances its ACT exp work (16.5us).
         v computed transposed as vT [seq, packed-c] with a leading ones
         column per head so the softmax denominator rides the av matmul
         (psum rows 0 / 64).
  attn:  per head pair, interleaved through the y-tile loop with a
         pair-shared av psum tile (rows 0:64 / 64:128, av j-major so the
         j0 half stops first) and THREE sT psum slots: sT[y,x] = k_h^T q_h
         (bf16, scale folded into wq) -> exp on ACT -> av accumulates
         u'[c,x] + denominator over y tiles in PSUM. ACT is ~gapless
         through pairs 1-3; PE is the binding stream (~80us busy).
  norm (pairs 0-2): denominator rows DMA-transposed to [128,16] via a DRAM
         bounce (DVE reciprocal is per-lane serial; staged yt2/yt3/yt5 of
         the next pair so the DVE queue never parks on a DMA wait),
         reciprocal in bf16, 1/den broadcast back with two stride-0 DMA
         reads (one per queue), one DVE multiply; off the critical path.
  norm (final pair): per j-half pipelined — av copy halves, den rows
         scatter-DMA'd to [128,8], DVE reciprocal, rows gathered to
         [2,512], then a K=2 selector MATMUL against a host-packed 0/1
         matrix broadcasts 1/den across partitions into PSUM (replaces the
         ~8us stride-0 broadcast DMA), one DVE multiply per half. Proj
         kt0-1 runs on the PE during this chain; kt2 fills the gap between
         the two selector matmuls.
  proj:  w_projT packed on K with zero pad rows (bf16); kt3 lands last per
         j-half; bias added during psum->sbuf copy on ACT (ot0/ot2) and
         DVE (ot1) in parallel; six bf16 output DMAs rotate across three
         queues; host casts back to f32.
"""

import sys

if "/opt/trn_rl_repo" not in sys.path:
    sys.path.insert(0, "/opt/trn_rl_repo")

import numpy as np

import concourse.bass as bass
import concourse.mybir as mybir
import concourse.tile as tile
from concourse import bacc
from concourse.bass_utils import run_bass_kernel_spmd

DIM = 384
HEADS = 8
DH = 48
SEQ = 1024
P = 128
NCORES = 8
VP = 64  # packed v cols per head: ones at col 0, zeros 1-15, 48 data at 16-63

F32 = mybir.dt.float32
F32R = mybir.dt.float32r
BF16 = mybir.dt.bfloat16
EXP = mybir.ActivationFunctionType.Exp
IDENT = mybir.ActivationFunctionType.Identity
ADD = mybir.AluOpType.add
MULT = mybir.AluOpType.mult
I16 = mybir.dt.int16
# bf16-space Schraudolph fast exp: bf16_bits(exp(x)) ~= int16(x*128/ln2 +
# (127*128 - 5.5)); max elementwise rel err ~3.3%, used on 12 of 64 softmax
# tiles where the softmax normalization washes most of it out (end-to-end
# rel err 7.8e-3 vs the 2e-2 gate, measured in numpy against the oracle)
FEXP_A = float(128.0 / np.log(2.0))
FEXP_B = 16256.0 - 5.5

_NC_CACHE = {}


def _emit(tc, nc, x_d, wq_d, wk_d, wv_d, wp_d, b_d, sel2_d, out_d):
    with (
        tc.tile_pool(name="const", bufs=1) as constp,
        tc.tile_pool(name="weights", bufs=1) as wpool,
        tc.tile_pool(name="data", bufs=1) as data,
        tc.tile_pool(name="ptile", bufs=8) as ppool,
        tc.tile_pool(name="bcpool", bufs=3) as bcpool,
        tc.tile_pool(name="rpool", bufs=3) as rpool,
        tc.tile_pool(name="avcp", bufs=2) as avcp,
        tc.tile_pool(name="opool", bufs=3) as opool,
        tc.tile_pool(name="ps_s", bufs=3, space="PSUM") as ps_s,
        tc.tile_pool(name="ps_av", bufs=1, space="PSUM") as ps_av,
        tc.tile_pool(name="dram", bufs=3, space="DRAM") as drampool,
    ):
        # ---- loads, ordered by need: x + wq/wk tile-0 cols gate the start
        x_sb = data.tile([P, 2, 3, 512], BF16, tag="x")
        wq_sb = wpool.tile([P, 3, 512], BF16, tag="wq")
        wk_sb = wpool.tile([P, 3, 512], BF16, tag="wk")
        wv_sb = wpool.tile([P, 3, HEADS * VP], BF16, tag="wv")
        wp_sb = wpool.tile([P, 4, DIM], BF16, tag="wp")
        bias_sb = constp.tile([P, 3], F32, tag="bias")

        # x packed j-half-major: each half is one 384KB DMA with 3KB lines
        nc.sync.dma_start(x_sb[:, 0], x_d.ap()[:, 0])
        nc.scalar.dma_start(x_sb[:, 1], x_d.ap()[:, 1])
        nc.gpsimd.dma_start(wq_sb[:, :, 0:128], wq_d.ap()[:, :, 0:128])
        nc.gpsimd.dma_start(wk_sb[:, :, 0:128], wk_d.ap()[:, :, 0:128])
        nc.sync.dma_start(wq_sb[:, :, 128:512], wq_d.ap()[:, :, 128:512])
        nc.scalar.dma_start(wk_sb[:, :, 128:512], wk_d.ap()[:, :, 128:512])
        nc.gpsimd.dma_start(wv_sb[:], wv_d.ap())
        nc.sync.dma_start(bias_sb[:], b_d.ap())
        nc.gpsimd.dma_start(wp_sb[:], wp_d.ap())

        zb_sb = constp.tile([P, 1], F32, tag="zb")
        nc.gpsimd.memset(zb_sb[:], 0.0)
        # selector for the final-pair 1/den broadcast matmul (host-packed):
        # bc[p, x] = sum_k sel2[k, p] * rec2[k, x] -> rows 0:64 get rec2[0],
        # rows 64:128 get rec2[1]
        sel2_sb = constp.tile([2, P], BF16, tag="sel2")
        nc.gpsimd.dma_start(sel2_sb[:], sel2_d.ap())

        q_sb = data.tile([P, 4, SEQ], BF16, tag="q")
        k_sb = data.tile([P, 4, SEQ], BF16, tag="k")
        vT_sb = data.tile([P, 8, HEADS, VP], BF16, tag="vT")
        u_sb = [data.tile([P, SEQ], BF16, tag=f"u{i}", name=f"u{i}") for i in range(4)]

        def emit_qk_tile(t, which):
            """6 matmuls + 1 cast: one q or k tile into bf16 SBUF."""
            dst, w = (q_sb, wq_sb) if which == "q" else (k_sb, wk_sb)
            ps = ps_s.tile([P, SEQ], F32, tag="s", name=f"{which}{t}_ps")
            for j in range(2):
                for ko in range(3):
                    nc.tensor.matmul(
                        ps[:, j * 512 : (j + 1) * 512],
                        lhsT=w[:, ko, t * 128 : (t + 1) * 128],
                        rhs=x_sb[:, j, ko, :],
                        start=(ko == 0),
                        stop=(ko == 2),
                    )
            nc.vector.tensor_copy(dst[:, t, :], ps[:])

        def emit_v_chunk(yt):
            """3 matmuls + copy + ones-column: vT rows for one y tile."""
            ps = ps_s.tile([P, SEQ], F32, tag="s", name="v_ps")
            for ko in range(3):
                nc.tensor.matmul(
                    ps[:, 0 : HEADS * VP],
                    lhsT=x_sb[:, yt // 4, ko, (yt % 4) * 128 : (yt % 4 + 1) * 128],
                    rhs=wv_sb[:, ko, :],
                    start=(ko == 0),
                    stop=(ko == 2),
                )
            nc.vector.tensor_copy(
                vT_sb[:, yt, :, :],
                ps[:, 0 : HEADS * VP].rearrange("p (h v) -> p h v", h=HEADS),
            )
            nc.gpsimd.memset(vT_sb[:, yt, :, 0:1], 1.0)

        # prelude: only q,k tile 0 before the pair loop, j-interleaved with
        # half-tile casts so sT(pair0, yt0) unblocks as early as possible
        q0_ps = ps_s.tile([P, SEQ], F32, tag="s", name="q0_ps")
        k0_ps = ps_s.tile([P, SEQ], F32, tag="s", name="k0_ps")
        for j in range(2):
            for ps, w, dst in ((q0_ps, wq_sb, q_sb), (k0_ps, wk_sb, k_sb)):
                for ko in range(3):
                    nc.tensor.matmul(
                        ps[:, j * 512 : (j + 1) * 512],
                        lhsT=w[:, ko, 0:128],
                        rhs=x_sb[:, j, ko, :],
                        start=(ko == 0),
                        stop=(ko == 2),
                    )
                nc.vector.tensor_copy(
                    dst[:, 0, j * 512 : (j + 1) * 512],
                    ps[:, j * 512 : (j + 1) * 512],
                )

        def epi_a(t, av_copy, q=None):
            # av_copy rows 0/64 = softmax denominators of heads 2t/2t+1.
            # DMA-transpose both to [128, 16] via a DRAM bounce so the
            # reciprocal runs on 128 lanes.
            q = q or nc.sync
            den_dram = drampool.tile([2 * SEQ], F32, tag="den")
            q.dma_start(den_dram[0:SEQ], av_copy[0:1, :])
            q.dma_start(den_dram[SEQ : 2 * SEQ], av_copy[64:65, :])
            den_pm = rpool.tile([P, 16], F32, tag="denpm")
            q.dma_start(den_pm[:], den_dram[:].rearrange("(p f) -> p f", p=P))
            rec_pm = rpool.tile([P, 16], BF16, tag="recpm")
            with nc.allow_low_precision(reason="softmax denom reciprocal to bf16"):
                nc.vector.reciprocal(rec_pm[:], den_pm[:])
            rec_dram = drampool.tile([2 * SEQ], BF16, tag="rec")
            q.dma_start(rec_dram[:], rec_pm[:])
            return rec_dram

        def epi_b_issue(t, av_copy, rec_dram):
            # stride-0 DMAs broadcast both heads' 1/denom to [128, 1024]
            # (rows 0:64 <- head 2t, rows 64:128 <- head 2t+1), one per queue
            # so the ~256KB broadcast lands in half the time.
            bc_sb = bcpool.tile([P, SEQ], BF16, tag="bcs")
            rec_r = rec_dram[:].rearrange("(h f) -> h f", h=2)
            nc.sync.dma_start(
                bc_sb[0:64, :], rec_r[0:1, None, :].to_broadcast([1, 64, SEQ])
            )
            nc.scalar.dma_start(
                bc_sb[64:128, :], rec_r[1:2, None, :].to_broadcast([1, 64, SEQ])
            )
            return bc_sb

        def epi_b_mult(t, av_copy, bc_sb):
            # emitted well after the broadcast issue so this never parks the
            # in-order DVE queue on a DMA wait.
            with nc.allow_low_precision(reason="bf16 u tile"):
                nc.vector.tensor_tensor(u_sb[t][:], av_copy[:], bc_sb[:], MULT)

        # Head-pair interleaved attention with qkv work spread through the
        # yt loops so the PE load per pair tracks the 16.5us ACT exp load.
        pending = []
        last_av = None
        for t in range(4):
            av_ps = ps_av.tile([P, SEQ], F32, tag="av", name=f"av{t}")
            p_tiles = [[None] * 8 for _ in range(2)]
            for yt in range(9):
                for s in range(2):
                    po = s * 64
                    if yt < 8:
                        sT_ps = ps_s.tile([P, SEQ], F32, tag="s", name="sT_ps")
                        for j in range(2):
                            nc.tensor.matmul(
                                sT_ps[:, j * 512 : (j + 1) * 512],
                                lhsT=k_sb[
                                    po : po + 48, t, yt * 128 : (yt + 1) * 128
                                ],
                                rhs=q_sb[po : po + 48, t, j * 512 : (j + 1) * 512],
                                start=True,
                                stop=True,
                            )
                        p_tiles[s][yt] = ppool.tile(
                            [P, SEQ], BF16, tag="p", name="p_sb"
                        )
                        if t > 0 and s == 1 and yt % 2 == 1:
                            # offload to the (otherwise idle) DVE: one
                            # multiply-add into an int16 view IS the bf16
                            # bit pattern of exp
                            with nc.allow_low_precision(reason="fast exp"):
                                nc.vector.tensor_scalar(
                                    p_tiles[s][yt][:].bitcast(I16),
                                    sT_ps[:],
                                    FEXP_A,
                                    FEXP_B,
                                    MULT,
                                    ADD,
                                )
                        else:
                            nc.scalar.activation(
                                p_tiles[s][yt][:], sT_ps[:], EXP, bias=zb_sb[:]
                            )
                if yt > 0:
                    for j in range(2):
                        for s in range(2):
                            po = s * 64
                            h = 2 * t + s
                            nc.tensor.matmul(
                                av_ps[po : po + VP, j * 512 : (j + 1) * 512],
                                lhsT=vT_sb[:, yt - 1, h, :],
                                rhs=p_tiles[s][yt - 1][:, j * 512 : (j + 1) * 512],
                                start=(yt == 1),
                                stop=(yt == 8),
                                skip_group_check=True,
                            )
                # qkv spread: v chunks ride pair 0; q/k tile t+1 rides pair t
                if t == 0 and yt < 8:
                    emit_v_chunk(yt)
                if t < 3:
                    if yt == 2:
                        emit_qk_tile(t + 1, "q")
                    elif yt == 4:
                        emit_qk_tile(t + 1, "k")
                # Epilogue staging (pair t-1, hosted early in pair t as in
                # the fast v2.3 schedule): reciprocal chain at yt2, broadcast
                # DMA issue at yt3, multiply at yt5 — the two-yt offset keeps
                # the in-order DVE queue from parking on the DMA wait.
                if yt == 2 and pending and pending[0][-1] == "avc":
                    tt, avc, _ = pending[0]
                    pending[0] = (tt, avc, epi_a(tt, avc), "rec")
                if yt == 3 and pending and pending[0][-1] == "rec":
                    tt, avc, rec, _ = pending[0]
                    pending[0] = (tt, avc, epi_b_issue(tt, avc, rec), "bc")
                if yt == 5 and pending and pending[0][-1] == "bc":
                    epi_b_mult(*pending.pop(0)[:3])
            av_copy = avcp.tile([P, SEQ], F32, tag="avc", name=f"avc{t}")
            if t < 3:
                nc.vector.tensor_copy(av_copy[:], av_ps[:])
                pending.append((t, av_copy, "avc"))
            else:
                last_av = (av_ps, av_copy)

        # ---- final pair epilogue, pipelined in j-halves ----
        # Per half: av copy out of PSUM, den rows scatter-transposed to
        # [128, 8] (rows 0:64 <- head A half, 64:128 <- head B half), DVE
        # reciprocal, rows gathered to [2, 512], K=2 selector matmul
        # broadcasts 1/den across partitions, one multiply. The j1 chain
        # runs one stage behind j0, and proj kt0-2 fills the PE meanwhile.
        av_ps3, av_copy3 = last_av
        halves = []
        for jh in range(2):
            sl = slice(jh * 512, (jh + 1) * 512)
            nc.vector.tensor_copy(av_copy3[:, sl], av_ps3[:, sl])
            den_pm = rpool.tile([P, 8], F32, tag="denpm", name=f"denpm{jh}")
            nc.sync.dma_start(den_pm[0:64, :], av_copy3[0:1, sl])
            nc.scalar.dma_start(den_pm[64:128, :], av_copy3[64:65, sl])
            halves.append((sl, den_pm))

        # proj kt0-1 for all three output tiles keeps the PE busy while the
        # final pair's 1/den chain resolves (emitted after av_copy so ot2 can
        # reuse the ps_av bank); kt2 is held back and emitted between the two
        # bc matmuls so the PE never idles (and never drops p-state) while
        # the rec2 gather DMAs land.
        pr_ps = []

        def emit_pr(kts):
            for ot in range(3):
                for j in range(2):
                    for kt in kts:
                        nc.tensor.matmul(
                            pr_ps[ot][:, j * 512 : (j + 1) * 512],
                            lhsT=wp_sb[:, kt, ot * 128 : (ot + 1) * 128],
                            rhs=u_sb[kt][:, j * 512 : (j + 1) * 512],
                            start=(kt == 0),
                            stop=False,
                        )

        for ot in range(3):
            pool = ps_s if ot < 2 else ps_av
            pr_ps.append(
                pool.tile(
                    [P, SEQ], F32, tag=("s" if ot < 2 else "av"), name=f"prps{ot}"
                )
            )
        emit_pr([0, 1])

        bc_ps = ps_s.tile([P, SEQ], F32, tag="s", name="bc_ps")
        for jh, (sl, den_pm) in enumerate(halves):
            rec_pm = rpool.tile([P, 8], BF16, tag="recpm", name=f"recpm{jh}")
            with nc.allow_low_precision(reason="softmax denom reciprocal to bf16"):
                nc.vector.reciprocal(rec_pm[:], den_pm[:])
            rec2_sb = rpool.tile([2, 512], BF16, tag="rec2", name=f"rec2_{jh}")
            nc.sync.dma_start(rec2_sb[0:1, :], rec_pm[0:64, :])
            nc.scalar.dma_start(rec2_sb[1:2, :], rec_pm[64:128, :])
            nc.tensor.matmul(
                bc_ps[:, sl],
                lhsT=sel2_sb[:, :],
                rhs=rec2_sb[:, :],
                start=True,
                stop=True,
            )
            if jh == 0:
                emit_pr([2])
            with nc.allow_low_precision(reason="bf16 u tile"):
                nc.vector.tensor_tensor(
                    u_sb[3][:, sl], av_copy3[:, sl], bc_ps[:, sl], MULT
                )

        # ---- proj kt3 + bias + output DMAs, per (ot, j-half) ----
        # ot1's bias add goes to the (idle) DVE so psum evacuations run on
        # two engines; six 128KB output DMAs rotate across three queues.
        oqs = [nc.sync, nc.scalar, nc.gpsimd]
        o_sbs = [opool.tile([P, SEQ], BF16, tag="o", name=f"o{ot}") for ot in range(3)]
        for jh in range(2):
            sl = slice(jh * 512, (jh + 1) * 512)
            for ot in range(3):
                nc.tensor.matmul(
                    pr_ps[ot][:, sl],
                    lhsT=wp_sb[:, 3, ot * 128 : (ot + 1) * 128],
                    rhs=u_sb[3][:, sl],
                    start=False,
                    stop=True,
                )
            for ot in range(3):
                o_sb = o_sbs[ot]
                if ot == 1:
                    with nc.allow_low_precision(reason="bf16 output tile"):
                        nc.vector.tensor_scalar(
                            o_sb[:, sl],
                            pr_ps[ot][:, sl],
                            bias_sb[:, ot : ot + 1],
                            None,
                            ADD,
                        )
                else:
                    nc.scalar.activation(
                        o_sb[:, sl],
                        pr_ps[ot][:, sl],
                        IDENT,
                        bias=bias_sb[:, ot : ot + 1],
                    )
                oqs[ot].dma_start(
                    out_d.ap()[ot * 128 : (ot + 1) * 128, sl], o_sb[:, sl]
                )


def build_nc():
    nc = bacc.Bacc("TRN2", target_bir_lowering=False, debug=False, num_devices=NCORES)
    x_d = nc.dram_tensor("x", [P, 2, 3, 512], BF16, kind="ExternalInput")
    wq_d = nc.dram_tensor("wq", [P, 3, 512], BF16, kind="ExternalInput")
    wk_d = nc.dram_tensor("wk", [P, 3, 512], BF16, kind="ExternalInput")
    wv_d = nc.dram_tensor("wv", [P, 3, HEADS * VP], BF16, kind="ExternalInput")
    wp_d = nc.dram_tensor("wp", [P, 4, DIM], BF16, kind="ExternalInput")
    b_d = nc.dram_tensor("bias", [P, 3], F32, kind="ExternalInput")
    sel2_d = nc.dram_tensor("sel2", [2, P], BF16, kind="ExternalInput")
    out_d = nc.dram_tensor("out", [DIM, SEQ], BF16, kind="ExternalOutput")

    with tile.TileContext(nc) as tc:
        _emit(tc, nc, x_d, wq_d, wk_d, wv_d, wp_d, b_d, sel2_d, out_d)
    nc.compile()
    return nc


import ml_dtypes

BF16NP = ml_dtypes.bfloat16


def pack_inputs(x, w_qkv, w_proj, b_proj):
    """Host-side weight packing. Returns per-core input maps."""
    x = np.asarray(x, np.float32)
    w_qkv = np.asarray(w_qkv, np.float32)
    w_proj = np.asarray(w_proj, np.float32)
    b_proj = np.asarray(b_proj, np.float32)
    scale = DH ** -0.5
    w_q, w_k, w_v = w_qkv[0:DIM], w_qkv[DIM : 2 * DIM], w_qkv[2 * DIM :]

    WQ = np.zeros((DIM, 512), np.float32)
    WK = np.zeros((DIM, 512), np.float32)
    WV = np.zeros((DIM, HEADS * VP), np.float32)
    WP = np.zeros((512, DIM), np.float32)
    for h in range(HEADS):
        col = (h // 2) * 128 + (h % 2) * 64
        WQ[:, col : col + DH] = (w_q[h * DH : (h + 1) * DH] * scale).T
        WK[:, col : col + DH] = w_k[h * DH : (h + 1) * DH].T
        WV[:, h * VP + 16 : h * VP + 16 + DH] = w_v[h * DH : (h + 1) * DH].T
        WP[col + 16 : col + 16 + DH, :] = w_proj[:, h * DH : (h + 1) * DH].T
    BIAS = np.ascontiguousarray(b_proj.reshape(3, P).T)

    def pm(a, chunks):
        # [(chunks*P), f] -> [P, chunks, f] partition-major pre-layout
        return np.ascontiguousarray(
            a.reshape(chunks, P, a.shape[-1]).transpose(1, 0, 2)
        )

    def pmb(a, chunks):
        return pm(a, chunks).astype(BF16NP)

    def xj(a):
        # [DIM, SEQ] -> [P, 2(j-half), 3(ko), 512] j-half-major bf16
        xp = pm(a, 3)  # [P, 3, SEQ]
        xp = xp.reshape(P, 3, 2, 512).transpose(0, 2, 1, 3)
        return np.ascontiguousarray(xp).astype(BF16NP)

    WQp, WKp, WVp, WPp = pmb(WQ, 3), pmb(WK, 3), pmb(WV, 3), pmb(WP, 4)
    SEL2 = np.zeros((2, P), np.float32)
    SEL2[0, 0:64] = 1.0
    SEL2[1, 64:128] = 1.0
    SEL2 = SEL2.astype(BF16NP)
    in_maps = []
    for b in range(NCORES):
        in_maps.append(
            {
                "x": xj(x[b].reshape(DIM, SEQ)),
                "wq": WQp,
                "wk": WKp,
                "wv": WVp,
                "wp": WPp,
                "bias": BIAS,
                "sel2": SEL2,
            }
        )
    return in_maps


def run(in_maps, trace=False):
    if "nc" not in _NC_CACHE:
        _NC_CACHE["nc"] = build_nc()
    nc = _NC_CACHE["nc"]
    res = run_bass_kernel_spmd(
        nc, in_maps, core_ids=list(range(NCORES)), trace=trace
    )
    out = np.stack(
        [np.asarray(res.results[i]["out"], np.float32) for i in range(NCORES)]
    )
    return out.reshape(NCORES, DIM, 32, 32), res


def kernel(x, w_qkv, w_proj, b_proj):
    out, _ = run(pack_inputs(x, w_qkv, w_proj, b_proj))
    return out
